# revision 8
# baseline (speedup 1.0000x reference)
"""Trainium2 Bass kernel: single-head causal attention, bf16 datapath.

Problem: x[B=8,T=2048,C=1024] @ Wq/Wk/Wv[C,H=64] -> causal softmax attention
-> out[B,T,H].  Sharding: data-parallel over B, one batch element per core
(8 cores, no collectives).

Design (driven by measured ISA penalties):
  - matmuls with <128 contraction partitions cost +~120ns; matmuls with >64
    moving cols but <128 out partitions, or 65-col movings, cost +~70-120ns.
    So every matmul here uses 128 contraction partitions and full-rate
    shapes:
      * scores: stationary kT zero-padded to [128,128] (rows 64-127 = 0.0,
        moving qkT junk rows x 0 = 0), moving = full [128,w] qkT tile.
      * att@v: v-stationary [128k, 128] with cols 0-63 = v, col 64 = ones
        (softmax denominator), cols 65-127 = 0; moving = et [128, w].
        Output accumulates as outT [65(:128), 512] per q-block; PE-transposed
        back per 128-tile, then scaled by 1/denom.
      * V projection: x-stationary [128c,128t] x Wv chunks [128c,64].
  - tight causal at 128-key granularity (17408 score rows + exp cols).
  - all operands bf16 (tol 2e-2, lands ~5e-3); host feeds xT/W bf16.
"""

import numpy as np

P = 128
B = 8
T = 2048
C = 1024
H = 64
QB = 512
NB = T // QB      # 4 q-blocks
CC = C // P       # 8 contraction chunks
KT = T // P       # 16 key tiles
N_CORES = 8

_CACHE = {}


def _build(reps=1):
    import concourse.bacc as bacc
    import concourse.mybir as mybir
    import concourse.tile as tile
    from concourse.masks import make_identity

    dt = mybir.dt
    f32 = dt.float32
    bf16 = dt.bfloat16
    AF = mybir.ActivationFunctionType
    ALU = mybir.AluOpType

    nc = bacc.Bacc(None, target_bir_lowering=False)
    xT_d = nc.dram_tensor("xT", [C, T], bf16, kind="ExternalInput")
    wqk_d = nc.dram_tensor("wqk", [C, 2 * H], bf16, kind="ExternalInput")
    wv_d = nc.dram_tensor("wv", [C, H], bf16, kind="ExternalInput")
    out_d = nc.dram_tensor("out", [T, H], f32, kind="ExternalOutput")

    with tile.TileContext(nc) as tc:
        with (
            tc.tile_pool(name="consts", bufs=1) as consts,
            tc.tile_pool(name="xpool", bufs=1) as xpool,
            tc.tile_pool(name="qkvp", bufs=1) as qkvp,
            tc.tile_pool(name="expp", bufs=32) as expp,
            tc.tile_pool(name="otp", bufs=2) as otp,
            tc.tile_pool(name="outp", bufs=1) as outp,
            tc.tile_pool(name="sclp", bufs=4) as sclp,
            tc.tile_pool(name="psA", bufs=2, space="PSUM") as psA,
            tc.tile_pool(name="psS", bufs=3, space="PSUM") as psS,
            tc.tile_pool(name="psV", bufs=1, space="PSUM") as psV,
            tc.tile_pool(name="psO", bufs=1, space="PSUM") as psO,
            tc.tile_pool(name="psT", bufs=1, space="PSUM") as psT,
        ):
            ident = consts.tile([P, P], bf16)
            make_identity(nc, ident)
            # trimask[p, c] = 1.0 if c >= p else 0.0  (valid where q >= k)
            trimask = consts.tile([P, P], bf16)
            nc.gpsimd.memset(trimask, 1.0)
            nc.gpsimd.affine_select(
                out=trimask,
                in_=trimask,
                compare_op=ALU.is_ge,
                fill=0.0,
                base=0,
                pattern=[[1, P]],
                channel_multiplier=-1,
            )

            wqk_sb = consts.tile([P, CC, 2 * H], bf16)
            nc.sync.dma_start(wqk_sb[:], wqk_d[:, :].rearrange("(c p) h -> p c h", p=P))
            wv_sb = consts.tile([P, CC, H], bf16)
            nc.sync.dma_start(wv_sb[:], wv_d[:, :].rearrange("(c p) h -> p c h", p=P))

            x_sb = xpool.tile([P, CC, T], bf16)
            for bb in range(NB):
                for c in range(CC):
                    nc.sync.dma_start(
                        x_sb[:, c, bb * QB:(bb + 1) * QB],
                        xT_d[c * P:(c + 1) * P, bb * QB:(bb + 1) * QB],
                    )

            # qkT rows 0-63 = q^T, rows 64-127 = k^T (junk for score movings)
            qkT_sb = qkvp.tile([P, T], bf16)
            # kT_pad rows 0-63 = k^T, rows 64-127 stay 0.0 (contraction pad)
            kT_pad = qkvp.tile([P, T], bf16)
            nc.vector.memset(kT_pad[H:P, :], 0.0)
            # v_stat[k, kc, 0:64] = v, [:, kc, 64] = 1.0, [:, kc, 65:128] = 0
            v_sb = qkvp.tile([P, KT, P], bf16)
            nc.vector.memset(v_sb[:], 0.0)
            ones_col = consts.tile([P, KT, 1], bf16)
            nc.gpsimd.memset(ones_col[:], 1.0)
            nc.vector.tensor_copy(v_sb[:, :, H:H + 1], ones_col[:])
            out_sb = outp.tile([P, KT, H], f32)

            def project_qk(b):
                bsl = slice(b * QB, (b + 1) * QB)
                ps = psA.tile([P, QB], f32, tag="a", name="ps_qk")
                for c in range(CC):
                    nc.tensor.matmul(
                        ps, wqk_sb[:, c, :], x_sb[:, c, bsl],
                        start=(c == 0), stop=(c == CC - 1),
                    )
                nc.vector.tensor_copy(qkT_sb[:, bsl], ps)
                nc.vector.tensor_copy(kT_pad[0:H, bsl], ps[H:P, :])

            def project_v(b):
                # x-stationary: [128c,128t] x Wv chunk [128c,64] -> v natural
                for s in range(4):
                    t = b * 4 + s
                    pv = psV.tile([P, H], f32, tag="v", name="ps_vn")
                    for c in range(CC):
                        nc.tensor.matmul(
                            pv,
                            x_sb[:, c, t * P:(t + 1) * P],
                            wv_sb[:, c, :],
                            start=(c == 0), stop=(c == CC - 1),
                        )
                    nc.vector.tensor_copy(v_sb[:, t, 0:H], pv)

            def scores(b, ets):
                for kc in range(4 * (b + 1)):
                    off = max(0, kc * P - b * QB)   # 128*j for diagonal tiles
                    w = QB - off
                    ps = psS.tile([P, QB], f32, tag="s", name="ps_s")
                    nc.tensor.matmul(
                        ps[:, 0:w],
                        kT_pad[:, kc * P:(kc + 1) * P],
                        qkT_sb[:, b * QB + off:(b + 1) * QB],
                    )
                    et = expp.tile([P, QB], bf16, tag="e", name="et")
                    nc.scalar.activation(et[:, 0:w], ps[:, 0:w], AF.Exp)
                    if off > 0 or kc == 4 * b:
                        nc.gpsimd.tensor_mul(et[:, 0:P], et[:, 0:P], trimask)
                    ets[kc] = (et, off)

            def attv(b, ets):
                # outT accumulate: po[0:64] = out^T, po[64] = denom
                po = psO.tile([P, QB], f32, tag="o", name="ps_o")
                for kc in range(4 * (b + 1)):
                    et, off = ets[kc]
                    w = QB - off
                    nc.tensor.matmul(
                        po[:, off:QB],
                        v_sb[:, kc, :],
                        et[:, 0:w],
                        start=(kc == 0),
                        stop=(kc == 4 * (b + 1) - 1),
                    )
                oT = otp.tile([H + 2, QB], bf16, tag="t", name="oT")
                for s in range(4):
                    i = b * 4 + s
                    # piecewise drain: copy 128 cols, transpose, scale --
                    # keeps the PE from waiting on one big PSUM read
                    nc.vector.tensor_copy(oT[:, s * P:(s + 1) * P],
                                          po[0:H + 2, s * P:(s + 1) * P])
                    pt = psT.tile([P, H + 2], bf16, tag="tt", name="ps_t")
                    nc.tensor.matmul(
                        pt,
                        oT[:, s * P:(s + 1) * P],
                        ident[0:H + 2, 0:H + 2],
                        is_transpose=True,
                    )
                    rc = sclp.tile([P, 1], f32, name="rc")
                    nc.vector.reciprocal(rc, pt[:, H:H + 1])
                    nc.vector.tensor_scalar_mul(out_sb[:, i, :], pt[:, 0:H], rc)

            def flush_out(b):
                nc.sync.dma_start(
                    out_d[:, :].rearrange("(g p) h -> p g h", p=P)[:, b * 4:(b + 1) * 4, :],
                    out_sb[:, b * 4:(b + 1) * 4, :],
                )

            def body():
                ets = [[None] * KT for _ in range(NB)]
                project_qk(0)
                project_v(0)
                scores(0, ets[0])
                for b in range(1, NB):
                    project_qk(b)
                    project_v(b)
                    attv(b - 1, ets[b - 1])
                    flush_out(b - 1)
                    scores(b, ets[b])
                attv(NB - 1, ets[NB - 1])
                flush_out(NB - 1)

            if reps == 1:
                body()
            else:
                with tc.For_i(0, reps):
                    body()

    nc.compile()
    return nc


def _get_nc():
    nc = _CACHE.get("nc")
    if nc is None:
        nc = _build()
        _CACHE["nc"] = nc
    return nc


def _make_in_maps(inputs):
    import ml_dtypes

    x = np.asarray(inputs["x"], dtype=np.float32)
    Wq = np.asarray(inputs["Wq"], dtype=np.float32)
    Wk = np.asarray(inputs["Wk"], dtype=np.float32)
    Wv = np.asarray(inputs["Wv"], dtype=np.float32)
    scale = np.float32(1.0 / np.sqrt(np.float32(Wq.shape[1])))
    bf = ml_dtypes.bfloat16
    wqk = np.ascontiguousarray(
        np.concatenate([Wq * scale, Wk], axis=1)).astype(bf)
    wv_c = np.ascontiguousarray(Wv).astype(bf)
    in_maps = []
    for b in range(N_CORES):
        in_maps.append({
            "xT": np.ascontiguousarray(x[b].T).astype(bf),
            "wqk": wqk,
            "wv": wv_c,
        })
    return in_maps


def _run(inputs, **kwargs):
    from concourse.bass_utils import run_bass_kernel_spmd

    nc = _get_nc()
    res = run_bass_kernel_spmd(nc, _make_in_maps(inputs), core_ids=list(range(N_CORES)), **kwargs)
    out = np.stack([res.results[i]["out"] for i in range(N_CORES)], axis=0)
    return out.astype(np.float32, copy=False), res


def kernel(**inputs):
    out, _ = _run(inputs)
    return out


def kernel_profiled(**inputs):
    out, res = _run(inputs)
    return out, res
